# revision 10
# baseline (speedup 1.0000x reference)
"""Trainium2 Bass kernel for nn_Colorizer (retrieval_knn).

v4 — Gram-matrix reformulation + all-bf16 (validated rel-err ~6e-3 vs
the 2e-2 gate on the fixed setup_inputs seed):

  S[r,t] = (W^T p_r)·(W^T p_t) = p_r^T (W W^T) p_t
so the per-image conv collapses to:
  1. G = W W^T            [192,192]   (768 PE cycles; W^T arrives as a
     [128, 2*192] two-chunk pack so each DMA row carries 768B — the
     naive [c,k] layout is 384B/row and descriptor-bound, ~3.2us/tile)
  2. gt = G @ p_tgt       [192,1024]  (4096 cycles; replaces the
     16384-cycle 4-image conv of the v1 kernel)
  3. S chunks: S[rc] = p_ref[:,rc]^T @ gt   (24 x 2048 cycles)
  4. E = exp(S - 50) on ACT, full [128,1024] tiles (one ACTIVATE costs
     ~260ns fixed + ~0.85ns/elem, so halves are net slower); -50 bias
     keeps fp32 exp in range, softmax is shift-invariant
  5. pred rows via lab_aug = [ones;zeros;labels] matmul (denominator
     rides along as rows 0..15), normalize in T-halves, DMA out fp32.

All matmuls bf16 (1 col/cycle at full clock). Inputs land bf16 (1.8
MB/core) over three DGE rings, ordered so every tensor arrives just
before its first use. PE work: 768 + 4096 + 24*(2048+1024) ~= 78.6k
cycles ~= 33 us at 2.4 GHz. Warm-up matmuls during the DMA prologue
start the HAM clock ramp (1.2 -> 2.4 GHz needs ~3us continuous work).

PSUM: one 3-buf pool of [128,1024] fp32 slots (6 banks) rotates
through warm-up/G/gt and the 24 S tiles — 3 chunks of lookahead so the
PE never stalls on ACT freeing a slot — plus 2 banks for pred.

Zero-padding: contract rows 64:128 of the k=128..191 chunk are zeroed
on BOTH operands (0 x garbage could be NaN). DVE ops are few and fat:
per-op fixed cost is ~0.5-0.7us, so pads are merged, not split.

Host side only reshapes/transposes/casts data; all FLOPs run on device.
Built on bacc.Bacc so compile() legalizes multi-semaphore waits.
"""

import numpy as np
from ml_dtypes import bfloat16

import concourse.mybir as mybir
from concourse import bacc
from concourse.bass_utils import run_bass_kernel_spmd
from concourse.tile import TileContext

F32 = mybir.dt.float32
BF16 = mybir.dt.bfloat16

N = 8            # samples == cores
R_T, T_T = 3, 1  # ref / target frames
H = W_IMG = 256
C = 3
PATCH = 8
FEAT = 256
K_LAB = 16
HP = H // PATCH          # 32
PPI = HP * HP            # 1024 patches per image
NIMG = R_T + T_T         # 4
NPAT = NIMG * PPI        # 4096
KPIX = PATCH * PATCH * C  # 192
R = R_T * PPI            # 3072
T = T_T * PPI            # 1024
RC = R // 128            # 24 r-chunks
LABC = 48                # 16 ones cols, 16 zero cols, 16 label cols
EXP_SHIFT = -50.0
N_WARMUP = 6
HT = T // 2              # 512-col halves


def _build_nc():
    nc = bacc.Bacc(trn_type="TRN2", target_bir_lowering=False)

    pt0_d = nc.declare_dram_parameter("pt0", [128, NPAT], BF16, isOutput=False)
    pt1_d = nc.declare_dram_parameter("pt1", [64, NPAT], BF16, isOutput=False)
    # wt_pack[p, j*KPIX+k] = W^T[j*128+p, k]  (c-chunk j in {0,1})
    wt_d = nc.declare_dram_parameter("wt", [128, 2 * KPIX], BF16, isOutput=False)
    lab_d = nc.declare_dram_parameter("lab", [128, RC * K_LAB], BF16, isOutput=False)
    out_d = nc.declare_dram_parameter("out", [K_LAB, T], F32, isOutput=True)

    with TileContext(nc) as tc:
        with (
            tc.tile_pool(name="const", bufs=1) as const,
            tc.tile_pool(name="mmps", bufs=3, space="PSUM") as mmps,
            tc.tile_pool(name="predps", bufs=1, space="PSUM") as predps,
            tc.tile_pool(name="epool", bufs=4) as epool,
            tc.tile_pool(name="opool", bufs=2) as opool,
        ):
            # PE warm-up source: first DVE op so matmuls can start early
            wu_sb = const.tile([128, 512], BF16, tag="wu")
            nc.vector.memset(wu_sb, 0.0)

            # ---- input loads over 3 DGE rings, earliest-need first ----
            wt_sb = const.tile([128, 2 * KPIX], BF16, tag="wt")
            pt0_sb = const.tile([128, NPAT], BF16, tag="pt0")
            pt1_sb = const.tile([128, NPAT], BF16, tag="pt1")
            lab_sb = const.tile([128, RC, LABC], BF16, tag="lab")

            # sync ring: wt lower half, then tgt halves, then ref b1, b2
            nc.sync.dma_start(out=wt_sb[64:128, :], in_=wt_d.ap()[64:128, :])
            nc.sync.dma_start(out=pt0_sb[:, 3072:3584], in_=pt0_d.ap()[:, 3072:3584])
            nc.sync.dma_start(out=pt0_sb[:, 3584:4096], in_=pt0_d.ap()[:, 3584:4096])
            nc.sync.dma_start(out=pt0_sb[:, 1024:2048], in_=pt0_d.ap()[:, 1024:2048])
            nc.sync.dma_start(out=pt0_sb[:, 2048:3072], in_=pt0_d.ap()[:, 2048:3072])
            # scalar ring: wt upper half, then ref b0 halves
            nc.scalar.dma_start(out=wt_sb[0:64, :], in_=wt_d.ap()[0:64, :])
            nc.scalar.dma_start(out=pt0_sb[:, 0:512], in_=pt0_d.ap()[:, 0:512])
            nc.scalar.dma_start(out=pt0_sb[:, 512:1024], in_=pt0_d.ap()[:, 512:1024])
            # gpsimd ring: pt1 tgt, pt1 ref b0, lab, pt1 ref b1, b2
            nc.gpsimd.dma_start(out=pt1_sb[0:64, 3072:4096], in_=pt1_d.ap()[:, 3072:4096])
            nc.gpsimd.dma_start(out=pt1_sb[0:64, 0:1024], in_=pt1_d.ap()[:, 0:1024])
            nc.gpsimd.dma_start(
                out=lab_sb[:, :, 32:48],
                in_=lab_d.ap().rearrange("p (rc k) -> p rc k", k=K_LAB),
            )
            nc.gpsimd.dma_start(out=pt1_sb[0:64, 1024:2048], in_=pt1_d.ap()[:, 1024:2048])
            nc.gpsimd.dma_start(out=pt1_sb[0:64, 2048:3072], in_=pt1_d.ap()[:, 2048:3072])
            nc.gpsimd.memset(lab_sb[:, :, 0:16].bitcast(BF16), 1.0)
            nc.gpsimd.memset(lab_sb[:, :, 16:32].bitcast(BF16), 0.0)

            # G rows 128:192 and gt rows 128:192 live in one tile so the
            # shared 64:128 zero-pad is a single DVE memset
            gg_sb = const.tile([128, KPIX + T], BF16, tag="gg")
            g_sb1 = gg_sb[:, 0:KPIX]
            gt_sb1 = gg_sb[:, KPIX:KPIX + T]
            g_sb0 = const.tile([128, KPIX], BF16, tag="g0")
            gt_sb0 = const.tile([128, T], BF16, tag="gt0")

            # DVE pad chain, need-ordered: gg (gt kc1 ~11.5), pt1 tgt pad
            # (gt kc1 rhs), pt1 b0 pad (S kc1 rc0 ~13), rest on gpsimd
            nc.vector.memset(gg_sb[64:128, :].bitcast(BF16), 0.0)
            nc.vector.memset(pt1_sb[64:128, 3072:4096].bitcast(BF16), 0.0)
            nc.vector.memset(pt1_sb[64:128, 0:1024].bitcast(BF16), 0.0)
            shift_sb = const.tile([128, 1], F32, tag="shift")
            nc.vector.memset(shift_sb, EXP_SHIFT)
            nc.gpsimd.memset(pt1_sb[64:128, 1024:3072].bitcast(BF16), 0.0)

            # ---- PE clock warm-up during the DMA prologue (HAM) ----
            for _ in range(N_WARMUP):
                wps = mmps.tile([128, 512], F32, tag="mm", name="wps")
                nc.tensor.matmul(wps, wu_sb[:, 0:128], wu_sb, start=True, stop=True)

            # ---- 1. G = W W^T (contract over FEAT as 2 packed 128-chunks) --
            g0_ps = mmps.tile([128, KPIX], F32, tag="mm", name="g0_ps")
            nc.tensor.matmul(
                g0_ps, wt_sb[:, 0:128], wt_sb[:, 0:KPIX], start=True, stop=False
            )
            nc.tensor.matmul(
                g0_ps, wt_sb[:, KPIX:KPIX + 128], wt_sb[:, KPIX:2 * KPIX],
                start=False, stop=True,
            )
            g1_ps = mmps.tile([128, KPIX], F32, tag="mm", name="g1_ps")
            nc.tensor.matmul(
                g1_ps[0:64, :], wt_sb[:, 128:KPIX], wt_sb[:, 0:KPIX],
                start=True, stop=False,
            )
            nc.tensor.matmul(
                g1_ps[0:64, :], wt_sb[:, KPIX + 128:2 * KPIX], wt_sb[:, KPIX:2 * KPIX],
                start=False, stop=True,
            )
            nc.scalar.copy(g_sb0, g0_ps)
            nc.scalar.copy(g_sb1[0:64, :], g1_ps[0:64, :])

            # ---- 2. gt = G @ p_tgt  [192, 1024] in two k1-chunks ----
            for k1c in range(2):
                sz = 128 if k1c == 0 else 64
                k1sl = slice(128 * k1c, 128 * k1c + sz)
                gt_ps = [
                    mmps.tile([128, HT], F32, tag="mm", name="gt_ps")
                    for _ in range(2)
                ]
                for ph in range(2):
                    tsl = slice(R + ph * HT, R + (ph + 1) * HT)
                    nc.tensor.matmul(
                        gt_ps[ph][0:sz, :], g_sb0[:, k1sl], pt0_sb[:, tsl],
                        start=True, stop=False,
                    )
                for ph in range(2):
                    tsl = slice(R + ph * HT, R + (ph + 1) * HT)
                    nc.tensor.matmul(
                        gt_ps[ph][0:sz, :], g_sb1[:, k1sl], pt1_sb[:, tsl],
                        start=False, stop=True,
                    )
                for ph in range(2):
                    psl = slice(ph * HT, (ph + 1) * HT)
                    dst = (gt_sb0 if k1c == 0 else gt_sb1)[0:sz, psl]
                    if k1c == 0:
                        nc.vector.tensor_copy(dst, gt_ps[ph][0:sz, :])
                    else:
                        nc.scalar.copy(dst, gt_ps[ph][0:sz, :])

            # ---- 3/4/5. S chunks -> exp -> pred accumulation ----
            pred_ps = predps.tile([LABC, T], F32, tag="pred")
            e_tiles = {}

            def s_part(rc):
                # S1,S2 share the pt0 chunk weights; S3,S4 share pt1;
                # pred(rc-2) sits between the pairs so its small 48-col
                # lab weight load hides under the S streams
                rsl = slice(rc * 128, (rc + 1) * 128)
                s_ps = mmps.tile([128, T], F32, tag="mm", name="s_ps")
                nc.tensor.matmul(
                    s_ps[:, 0:HT], pt0_sb[:, rsl], gt_sb0[:, 0:HT],
                    start=True, stop=False,
                )
                nc.tensor.matmul(
                    s_ps[:, HT:T], pt0_sb[:, rsl], gt_sb0[:, HT:T],
                    start=True, stop=False,
                )
                if rc >= PRED_LAG:
                    pred_part(rc - PRED_LAG)
                nc.tensor.matmul(
                    s_ps[:, 0:HT], pt1_sb[:, rsl], gt_sb1[:, 0:HT],
                    start=False, stop=True,
                )
                nc.tensor.matmul(
                    s_ps[:, HT:T], pt1_sb[:, rsl], gt_sb1[:, HT:T],
                    start=False, stop=True,
                )
                e_sb = epool.tile([128, T], BF16, tag="e", name="e_sb")
                if rc < RC - 1:
                    nc.scalar.activation(
                        e_sb, s_ps, mybir.ActivationFunctionType.Exp,
                        bias=shift_sb, scale=1.0,
                    )
                else:
                    # halves on the last chunk: h0's exp->pred->normalize
                    # chain starts ~0.5us earlier
                    for ph in range(2):
                        psl = slice(ph * HT, (ph + 1) * HT)
                        nc.scalar.activation(
                            e_sb[:, psl], s_ps[:, psl],
                            mybir.ActivationFunctionType.Exp,
                            bias=shift_sb, scale=1.0,
                        )
                e_tiles[rc] = e_sb

            def pred_part(rc):
                e_sb = e_tiles.pop(rc)
                for ph in range(2):
                    psl = slice(ph * HT, (ph + 1) * HT)
                    nc.tensor.matmul(
                        pred_ps[:, psl], lab_sb[:, rc, :], e_sb[:, psl],
                        start=(rc == 0), stop=(rc == RC - 1),
                    )

            PRED_LAG = 2
            for rc in range(RC):
                s_part(rc)
            for rc in range(RC - PRED_LAG, RC):
                pred_part(rc)

            # ---- normalize label rows in T-halves (overlaps pred drain) ----
            for ph in range(2):
                psl = slice(ph * HT, (ph + 1) * HT)
                rec = opool.tile([K_LAB, HT], F32, tag="rec", name="rec")
                nc.vector.reciprocal_approx_fast(rec, pred_ps[0:K_LAB, psl])
                o_sb = opool.tile([K_LAB, HT], F32, tag="o", name="o_sb")
                nc.vector.tensor_mul(o_sb, pred_ps[32:32 + K_LAB, psl], rec)
                nc.sync.dma_start(out=out_d.ap()[:, psl], in_=o_sb)

    nc.compile()
    return nc


_NC_CACHE = None


def _get_nc():
    global _NC_CACHE
    if _NC_CACHE is None:
        _NC_CACHE = _build_nc()
    return _NC_CACHE


def prep_in_maps(reference_images, target_images, reference_labels, w_feat):
    """Host-side sharding + layout prep (reshape/transpose/cast only)."""
    ri = np.ascontiguousarray(reference_images, dtype=np.float32)
    ti = np.ascontiguousarray(target_images, dtype=np.float32)
    lab = np.ascontiguousarray(reference_labels, dtype=np.float32)
    wf = np.ascontiguousarray(w_feat, dtype=np.float32)

    wT = np.ascontiguousarray(wf.reshape(KPIX, FEAT).T)       # [256, 192]
    wt_pack = np.concatenate([wT[0:128], wT[128:256]], axis=1)  # [128, 384]
    wt_pack = np.ascontiguousarray(wt_pack).astype(bfloat16)
    imgs = np.concatenate([ri, ti], axis=1)  # [N, 4, H, W, C]
    # patchesT[n] : [(dy dx ch), (img py px)]
    ptT = np.ascontiguousarray(
        imgs.reshape(N, NIMG, HP, PATCH, HP, PATCH, C)
        .transpose(0, 3, 5, 6, 1, 2, 4)
        .reshape(N, KPIX, NPAT)
    ).astype(bfloat16)
    lab_sw = np.ascontiguousarray(
        lab.reshape(N, RC, 128, K_LAB).transpose(0, 2, 1, 3).reshape(N, 128, RC * K_LAB)
    ).astype(bfloat16)
    return [
        {
            "pt0": np.ascontiguousarray(ptT[n][0:128]),
            "pt1": np.ascontiguousarray(ptT[n][128:KPIX]),
            "wt": wt_pack,
            "lab": lab_sw[n],
        }
        for n in range(N)
    ]


def run(in_maps, **kwargs):
    nc = _get_nc()
    return run_bass_kernel_spmd(nc, in_maps, list(range(N)), **kwargs)


def kernel(reference_images, target_images, reference_labels, w_feat):
    in_maps = prep_in_maps(
        reference_images, target_images, reference_labels, w_feat
    )
    res = run(in_maps)
    # device emits [16, T]; transpose to [T, 16] here (pure layout)
    out = np.stack(
        [np.ascontiguousarray(res.results[n]["out"].T) for n in range(N)]
    )
    return out.reshape(N, T_T, HP, HP, K_LAB)
